# revision 58
# baseline (speedup 1.0000x reference)
"""Multi-head attention forward on 8 Trainium2 NeuronCores.

Strategy: pure data-parallel over batch (B=8 -> 1 batch element per core,
no collectives). Per core, one fused kernel computes
    y = softmax((x Wq + bq)(x Wk + bk)^T / sqrt(hd)) (x Wv + bv) @ Wp + bp
for x [1024, 768], H=12 heads of 64 dims.

Layout choices (all matmuls contract over the SBUF partition dim):
  - x^T [768, 1024] built from f32 x via PE transposes (f32 in, bf16 out in
    the PSUM drain on DVE).
  - Q^T/K^T computed in "dout-major" layout [1536, 1024] (12 tiles of 128
    partitions = 2 heads each), interleaved m-order so head 0 unblocks early.
  - V computed in s-major layout [1024, 12*65] with a constant-1 column per
    head, so each AV matmul also produces the softmax denominator row.
  - scores^T [k, q] per head; the two heads of a pair run as CONCURRENT
    64-row PE tiles (tile_position (0,0)/(64,0)) -> 2x scores throughput.
    exp on ScalarE with the 1/8 scale folded in.
  - AV: out_h^T [65, q] = V_ext^T @ exp^T accumulated over k tiles; row 64
    holds the softmax sums Z. Sums are staged to DRAM; 1/Z = exp(-ln Z)
    batched on ScalarE, lifted back and broadcast with ONE full-mode matmul
    per (pair, q-half) via a constant selector matrix (no PE mode switches).
  - Bias adds for V / proj are full-mode matmuls against zero-padded bias
    tiles (row 0 = bias) so the PE never drops into 32-row tiling mode.
  - The whole kernel is software-pipelined: pair g's scores/exp overlap
    pair g-1's AV and pair g+1's Q/K projection so the TensorEngine stays
    dense while ScalarE chews on exp (12.6M transcendentals).
  - Startup: weight pair-0 + x DMAs issue first; dummy matmuls warm the PE
    clock gate (HAM) while DMAs are in flight.
Compute dtype bf16 (fp32 PSUM accumulation).
"""

import sys

for _p in ("/opt/trn_rl_repo", "/root/.axon_site/_ro/trn_rl_repo"):
    if _p not in sys.path:
        sys.path.append(_p)

import numpy as np

import concourse.bacc as bacc
import concourse.mybir as mybir
import concourse.tile as tile
from concourse.bass_utils import run_bass_kernel_spmd
from concourse.masks import make_identity

N_CORES = 8
P = 128
S = 1024
D = 768
H = 12
HD = 64
ND = D // P            # 6 d_model chunks
NS = S // P            # 8 seq tiles
NM = (2 * D) // P      # 12 M-tiles over Q,K douts
SCALE = 1.0 / (HD ** 0.5)
BF = mybir.dt.bfloat16
F32 = mybir.dt.float32
AF = mybir.ActivationFunctionType
ALU = mybir.AluOpType

_cached = None


def _patch_act_tables():
    """Force every Exp/Ln activation onto the one table set that holds both
    (`natural_log_exp_and_others`), so the table is loaded once instead of
    thrashing between `exp_and_others` and the ln set on every head."""
    import concourse.bacc as _bacc
    if getattr(_bacc, "_act_tables_patched", False):
        return
    orig = _bacc.get_activation_tables

    def patched(arch):
        tables = dict(orig(arch))
        for name, fns in tables.items():
            if name != "natural_log_exp_and_others":
                tables[name] = fns - {AF.Exp, AF.Ln}
        return tables

    _bacc.get_activation_tables = patched
    _bacc._act_tables_patched = True


def _build():
    _patch_act_tables()
    nc = bacc.Bacc("TRN2", target_bir_lowering=False, debug=False,
                   enable_asserts=True, num_devices=N_CORES)

    x_ext = nc.dram_tensor("x", [S, D], F32, kind="ExternalInput").ap()
    wq_ext = nc.dram_tensor("W_qkv", [D, 3 * D], F32, kind="ExternalInput").ap()
    bq_ext = nc.dram_tensor("b_qkv", [1, 3 * D], F32, kind="ExternalInput").ap()
    wp_ext = nc.dram_tensor("W_proj", [D, D], F32, kind="ExternalInput").ap()
    bp_ext = nc.dram_tensor("b_proj", [1, D], F32, kind="ExternalInput").ap()
    out_ext = nc.dram_tensor("out", [S, D], F32, kind="ExternalOutput").ap()

    with tile.TileContext(nc) as tc:
        _body(nc, tc, x_ext, wq_ext, bq_ext, wp_ext, bp_ext, out_ext)

    nc.compile()
    return nc


def _body(nc, tc, x_ext, wq_ext, bq_ext, wp_ext, bp_ext, out_ext):
    from contextlib import ExitStack
    from concourse.tile import add_dep_helper
    with ExitStack() as ctx:
        persist = ctx.enter_context(tc.tile_pool(name="persist", bufs=1))
        yout = ctx.enter_context(tc.tile_pool(name="yout", bufs=2))
        ps_mm = ctx.enter_context(tc.tile_pool(name="ps_mm", bufs=2, space="PSUM"))

        # identity first (gpsimd affine_select is brief), then weight DMAs
        # on the same gpsimd (SWDGE) ring so the startup-critical Q/K pair-0
        # columns are in flight early.  All W loads are f32->bf16 cast-DMAs.
        ident = persist.tile([P, P], F32)
        make_identity(nc, ident)
        w_bf = persist.tile([P, ND, 3 * D], BF)
        # Q/K columns per-kc (6KB descriptors -- DMA cost is per descriptor,
        # so big contiguous per-partition runs matter more than pair order),
        # then V bias, V columns, proj bias, proj weights.  SWDGE ring FIFO
        # paces the transfers; no explicit deps.
        wqk_last = None
        for kc in range(ND):
            wqk_last = nc.gpsimd.dma_start(
                w_bf[:, kc, 0:2 * D], wq_ext[kc * P:(kc + 1) * P, 0:2 * D])
        wv_last = None
        for kc in range(ND):
            wv_last = nc.gpsimd.dma_start(
                w_bf[:, kc, 2 * D:3 * D],
                wq_ext[kc * P:(kc + 1) * P, 2 * D:3 * D])
            add_dep_helper(wv_last.ins, wqk_last.ins,
                           reason="V weights after QK weights")
        bv_pad = persist.tile([P, D], BF)
        bp_pad = persist.tile([P, D], BF)
        nc.vector.memset(bv_pad, 0.0)
        nc.vector.memset(bp_pad, 0.0)
        d = nc.gpsimd.dma_start(bv_pad[0:1, :], bq_ext[0:1, 2 * D:3 * D])
        add_dep_helper(d.ins, wqk_last.ins, reason="bv after QK")
        d = nc.gpsimd.dma_start(bp_pad[0:1, :], bp_ext[0:1, :])
        add_dep_helper(d.ins, wv_last.ins, reason="bp after V")
        wp_bf = persist.tile([P, ND, D], BF)     # row chunk g = head pair g
        for g in range(ND):
            wp_dma = nc.gpsimd.dma_start(wp_bf[:, g, :],
                                         wp_ext[g * P:(g + 1) * P, :])
            add_dep_helper(wp_dma.ins, wv_last.ins,
                           reason="proj weights after V weights")

        # ---- x loads on both HWDGE rings (f32, no cast; PE transposes read
        # f32 directly and the DVE drain casts to bf16) ----
        warm = persist.tile([P, 640], BF)
        nc.vector.memset(warm, 0.5)
        xT = persist.tile([P, ND, S], BF)
        with tc.tile_pool(name="xin", bufs=6) as xin, \
             tc.tile_pool(name="ps_tr", bufs=6, space="PSUM") as ps_tr:
            x_tiles = []
            bqkT = persist.tile([P, NM], F32)  # col m = b_qkv[m*128:(m+1)*128]
            for sb in range(NS):
                x_f = xin.tile([P, D], F32, tag="x_f")
                if sb < 6:
                    eng = nc.sync if sb % 2 == 0 else nc.scalar
                    eng.dma_start(x_f, x_ext[sb * P:(sb + 1) * P, :])
                else:
                    # sb6-7 are needed only after the first QKV groups; keep
                    # their descriptors out of the DMA queues until the
                    # startup-critical QK weight transfer has finished
                    d = nc.sync.dma_start(x_f, x_ext[sb * P:(sb + 1) * P, :])
                    add_dep_helper(d.ins, wqk_last.ins,
                                   reason="late x after QK weights")
                x_tiles.append(x_f)
                if sb == 3:  # QKV biases right behind the first four x tiles
                    for m in range(NM):
                        nc.sync.dma_start(bqkT[:, m:m + 1],
                                          bq_ext[0:1, m * P:(m + 1) * P])

            for sb in range(NS):
                x_f = x_tiles[sb]
                for kc in range(ND):
                    pt = ps_tr.tile([P, 512], F32, tag="ps_tr",
                                    name=f"tr{sb}_{kc}")
                    nc.tensor.transpose(pt[:, 0:P], x_f[:, kc * P:(kc + 1) * P],
                                        ident)
                    nc.vector.tensor_copy(xT[:, kc, sb * P:(sb + 1) * P],
                                          pt[:, 0:P])
        expp = ctx.enter_context(tc.tile_pool(name="expp", bufs=32))
        sums_p = ctx.enter_context(tc.tile_pool(name="sums", bufs=1))
        ps_sc = ctx.enter_context(tc.tile_pool(name="ps_sc", bufs=2, space="PSUM"))
        ps_av = ctx.enter_context(tc.tile_pool(name="ps_av", bufs=2, space="PSUM"))

        # row0_mat: row 0 all ones (full-mode bias broadcast lhsT)
        row0_mat = persist.tile([P, P], BF)
        nc.vector.memset(row0_mat, 0.0)
        nc.vector.memset(row0_mat[0:1, :], 1.0)
        # sel2: (row0, cols 0:64)=1 and (row32, cols 64:128)=1 -- lifts the
        # two 1/Z rows of rec2 to partition blocks 0:64/64:128 in one
        # full-mode matmul
        sel2 = persist.tile([P, P], BF)
        nc.vector.memset(sel2, 0.0)
        nc.vector.memset(sel2[0:1, 0:HD], 1.0)
        nc.vector.memset(sel2[32:33, HD:P], 1.0)
        vext = persist.tile([P, NS, H * 65], BF)
        for sb in range(NS):
            vd = vext[:, sb, :].rearrange("p (h c) -> p h c", c=65)
            nc.vector.memset(vd[:, :, 64:65], 1.0)
        # zt: softmax sums of the pair in flight (row 0/32 = head half,
        # slot=g%2); rec2: 1/Z rows.  zt is all-ones so Ln/Exp over rows
        # 0:33 gives 1.0 in the unused rows; sel2 zeros there kill them.
        zt = persist.tile([P, 2, S], BF)
        nc.vector.memset(zt, 1.0)   # ln(1)=0 -> 1/Z rows stay finite
        rec2 = persist.tile([P, 2, S], BF)
        nc.vector.memset(rec2, 0.0)

        qkT = persist.tile([P, NM, S], BF)
        aoT = persist.tile([P, ND, S], BF)   # paired attn out^T: pair g rows

        import itertools

        def gen_qkT(g):
            """Q^T/K^T tiles for pair g; yields each PE instruction (or None
            for non-PE work) so stage() can chain PE queue order.
            Order (Q,nh0),(K,nh0),(Q,nh1),(K,nh1) so the first scores
            matmuls of the pair unblock after two groups."""
            for m, nh in ((g, 0), (ND + g, 0), (g, 1), (ND + g, 1)):
                if True:
                    ps = ps_mm.tile([P, 512], F32, tag="ps_mm",
                                    name=f"qk{m}_{nh}")
                    for kc in range(ND):
                        mm = nc.tensor.matmul(
                            ps, w_bf[:, kc, m * P:(m + 1) * P],
                            xT[:, kc, nh * 512:(nh + 1) * 512],
                            start=(kc == 0), stop=(kc == ND - 1))
                        yield mm
                    nc.vector.tensor_scalar(
                        out=qkT[:, m, nh * 512:(nh + 1) * 512], in0=ps,
                        scalar1=bqkT[:, m:m + 1], scalar2=None, op0=ALU.add)
                    yield None

        def gen_v():
            """V in s-major with ones column per head."""
            for sb in range(NS):
                for c0, cn in ((0, 512), (512, 256)):
                    ps = ps_mm.tile([P, 512], F32, tag="ps_mm",
                                    name=f"v{sb}_{c0}")
                    for kc in range(ND):
                        mm = nc.tensor.matmul(
                            ps[:, :cn], xT[:, kc, sb * P:(sb + 1) * P],
                            w_bf[:, kc, 2 * D + c0:2 * D + c0 + cn],
                            start=(kc == 0), stop=False)
                        yield mm
                    mm = nc.tensor.matmul(ps[:, :cn], row0_mat,
                                          bv_pad[:, c0:c0 + cn],
                                          start=False, stop=True)
                    yield mm
                    h0 = c0 // HD
                    nh_h = cn // HD
                    vsrc = ps[:, :cn].rearrange("p (h c) -> p h c", c=HD)
                    vdst = vext[:, sb, :].rearrange("p (h c) -> p h c", c=65)
                    nc.vector.tensor_copy(vdst[:, h0:h0 + nh_h, 0:HD], vsrc)
                    yield None

        def av_epilogue(g, half, qh, po):
            """One PSUM read frees po; the Z row goes to zt (partition
            half, slot g%2); values relayed to aoT via a fast SBUF->SBUF
            bf16 copy."""
            h = 2 * g + half
            rows = slice(half * HD, (half + 1) * HD)
            qs = slice(qh * 512, (qh + 1) * 512)
            sst = sums_p.tile([65, 512], BF, tag="sst", name=f"sst{h}_{qh}",
                              bufs=6)
            nc.vector.tensor_copy(sst, po)
            nc.vector.tensor_copy(zt[32 * half:32 * half + 1, g % 2, qs],
                                  sst[64:65, :])
            nc.vector.tensor_copy(aoT[rows, g, qs], sst[0:64, :])
            return sst

        def finish_av(g, exps, interleaved, sums4):
            """Emit whatever AV work for pair g was not interleaved,
            kb-outer with both q-halves sharing each V weight load."""
            for half in range(2):
                todo = [qh for qh in range(2) if (half, qh) not in interleaved]
                if not todo:
                    continue
                h = 2 * g + half
                pos = {}
                for qh in todo:
                    pos[qh] = ps_av.tile([65, 512], F32, tag="ps_av",
                                         name=f"po{h}_{qh}x")
                for kb in range(NS):
                    for qh in todo:
                        nc.tensor.matmul(
                            pos[qh],
                            vext[:, kb, h * 65:(h + 1) * 65],
                            exps[half][kb][:, qh * 512:(qh + 1) * 512],
                            start=(kb == 0), stop=(kb == NS - 1))
                for qh in todo:
                    sums4[(half, qh)] = av_epilogue(g, half, qh, pos[qh])

        def stage(g, fillers, prev_exps, n_interleave=2, n_fill=6,
                  norm_g=None):
            """Scores+exp for pair g, with the previous pair's AV and other
            PE work threaded between the kb steps so the PE never starves
            while ScalarE chews on exp.  The 4 scores matmuls per kb are a
            dep-chained T0/T8/T0/T8 quartet so the two 64-row PE tiles run
            concurrently."""
            AVSET = ((0, 0), (0, 1), (1, 0))[:n_interleave]
            po = {}
            sums4 = {}
            e0 = []
            e1 = []
            acts = []
            # chain = force the exact PE queue order (AVs -> fillers ->
            # scores quartet per kb) so the T0/T8 quartet MMs stay adjacent
            # and pair up on the array.  Only for steady-state stages --
            # stage 0's fillers may stall on weight DMAs.
            chain = False
            pe_last = None

            def link(mm):
                nonlocal pe_last
                if chain and pe_last is not None:
                    add_dep_helper(mm.ins, pe_last.ins, reason="pe order")
                pe_last = mm
                return mm

            for kb in range(NS):
                if prev_exps is not None:
                    for half, qh in AVSET:
                        h = 2 * (g - 1) + half
                        if kb == 0:
                            po[(half, qh)] = ps_av.tile(
                                [65, 512], F32, tag="ps_av",
                                name=f"po{h}_{qh}")
                        link(nc.tensor.matmul(
                            po[(half, qh)],
                            vext[:, kb, h * 65:(h + 1) * 65],
                            prev_exps[half][kb][:, qh * 512:(qh + 1) * 512],
                            start=(kb == 0), stop=(kb == NS - 1)))
                for _ in range(n_fill):
                    r = next(fillers, StopIteration)
                    if r is StopIteration:
                        break
                    if r is not None:
                        link(r)
                ps0 = ps_sc.tile([P, S], F32, tag="ps_sc", name=f"sc0_{g}_{kb}")
                ps1 = ps_sc.tile([P, S], F32, tag="ps_sc", name=f"sc1_{g}_{kb}")
                kbs = slice(kb * P, (kb + 1) * P)
                for qh in range(2):
                    qs = slice(qh * 512, (qh + 1) * 512)
                    nc.tensor.matmul(ps0[:, qs], qkT[0:HD, ND + g, kbs],
                                     qkT[0:HD, g, qs], start=True, stop=True,
                                     tile_position=(0, 0))
                    nc.tensor.matmul(ps1[:, qs], qkT[HD:P, ND + g, kbs],
                                     qkT[HD:P, g, qs], start=True, stop=True,
                                     tile_position=(64, 0))
                t0 = expp.tile([P, S], BF, tag="expT", name=f"e0_{g}_{kb}")
                t1 = expp.tile([P, S], BF, tag="expT", name=f"e1_{g}_{kb}")
                a0 = nc.scalar.activation(t0, ps0, AF.Exp, scale=SCALE)
                a1 = nc.scalar.activation(t1, ps1, AF.Exp, scale=SCALE)
                acts.append((a0, a1))
                e0.append(t0)
                e1.append(t1)
            if prev_exps is not None:
                for half, qh in AVSET:
                    sums4[(half, qh)] = av_epilogue(g - 1, half, qh,
                                                    po[(half, qh)])
                finish_av(g - 1, prev_exps, set(AVSET), sums4)
            return (e0, e1)

        def norm_scalar(g):
            """Batched ln/exp for pair g's softmax sums (rows 0/32 of zt)."""
            slot = g % 2
            lnz = sums_p.tile([33, S], F32, tag="lnz", name=f"lnz{g}",
                              bufs=2)
            nc.scalar.activation(lnz, zt[0:33, slot, :], AF.Ln)
            nc.scalar.activation(rec2[0:33, slot, :], lnz, AF.Exp,
                                 scale=-1.0)

        def norm_pb(g, link_fn):
            """Lift the 1/Z rows to partition blocks with one full-mode
            matmul per q-half and scale aoT."""
            slot = g % 2
            for qh in range(2):
                qs = slice(qh * 512, (qh + 1) * 512)
                pb = ps_mm.tile([P, 512], F32, tag="ps_mm",
                                name=f"pb{g}_{qh}")
                link_fn(nc.tensor.matmul(pb, sel2, rec2[:, slot, qs],
                                         start=True, stop=True))
                nc.vector.tensor_mul(aoT[:, g, qs], aoT[:, g, qs], pb)

        # --- pipeline ---
        for _ in gen_qkT(0):
            pass
        fill0 = itertools.chain(gen_v(), gen_qkT(1))
        exps_prev = stage(0, fill0, None, n_fill=20)
        for _ in fill0:
            pass
        for g in range(1, ND):
            fill = gen_qkT(g + 1) if g + 1 < ND else iter(())
            exps_new = stage(g, fill, exps_prev)
            for _ in fill:
                pass
            exps_prev = exps_new
            # normalize the pair finished inside this stage; the scheduler
            # hides it under the next stage's compute
            norm_scalar(g - 1)
            norm_pb(g - 1, lambda mm: mm)
        sums5 = {}
        finish_av(ND - 1, exps_prev, set(), sums5)
        norm_scalar(ND - 1)
        norm_pb(ND - 1, lambda mm: mm)

        # ---- output projection (paired K=128 chunks); both column
        # halves accumulate in one 2-bank tile from the now-idle scores
        # pool, so each aoT weight load feeds N=768 of streaming ----
        for sb in range(NS):
            y_sb = yout.tile([P, D], F32, tag="y")
            ps = ps_sc.tile([P, S], F32, tag="ps_sc", name=f"prj{sb}")
            for g in range(ND):
                nc.tensor.matmul(ps[:, 0:512],
                                 aoT[:, g, sb * P:(sb + 1) * P],
                                 wp_bf[:, g, 0:512],
                                 start=(g == 0), stop=False)
                nc.tensor.matmul(ps[:, 512:768],
                                 aoT[:, g, sb * P:(sb + 1) * P],
                                 wp_bf[:, g, 512:768],
                                 start=(g == 0), stop=False)
            nc.tensor.matmul(ps[:, 0:512], row0_mat, bp_pad[:, 0:512],
                             start=False, stop=True)
            nc.tensor.matmul(ps[:, 512:768], row0_mat, bp_pad[:, 512:768],
                             start=False, stop=True)
            nc.vector.tensor_copy(y_sb, ps[:, 0:D])
            nc.sync.dma_start(out_ext[sb * P:(sb + 1) * P, :], y_sb)


def kernel(**inputs):
    global _cached
    x = np.ascontiguousarray(np.asarray(inputs["x"], dtype=np.float32))
    w_qkv = np.ascontiguousarray(np.asarray(inputs["W_qkv"], dtype=np.float32))
    b_qkv = np.ascontiguousarray(np.asarray(inputs["b_qkv"], dtype=np.float32)).reshape(1, -1)
    w_proj = np.ascontiguousarray(np.asarray(inputs["W_proj"], dtype=np.float32))
    b_proj = np.ascontiguousarray(np.asarray(inputs["b_proj"], dtype=np.float32)).reshape(1, -1)

    if _cached is None:
        _cached = _build()
    nc = _cached

    in_maps = [{"x": x[b], "W_qkv": w_qkv, "b_qkv": b_qkv,
                "W_proj": w_proj, "b_proj": b_proj} for b in range(N_CORES)]
    last_err = None
    for _attempt in range(3):
        try:
            res = run_bass_kernel_spmd(nc, in_maps,
                                       core_ids=list(range(N_CORES)))
            return np.stack([res.results[i]["out"] for i in range(N_CORES)],
                            axis=0)
        except Exception as e:  # transient NRT device errors happen rarely
            last_err = e
            import time
            time.sleep(2.0)
    raise last_err


# revision 59
# speedup vs baseline: 1.0440x; 1.0440x over previous
"""Multi-head attention forward on 8 Trainium2 NeuronCores.

Strategy: pure data-parallel over batch (B=8 -> 1 batch element per core,
no collectives). Per core, one fused kernel computes
    y = softmax((x Wq + bq)(x Wk + bk)^T / sqrt(hd)) (x Wv + bv) @ Wp + bp
for x [1024, 768], H=12 heads of 64 dims.

Layout choices (all matmuls contract over the SBUF partition dim):
  - x^T [768, 1024] built from f32 x via PE transposes (f32 in, bf16 out in
    the PSUM drain on DVE).
  - Q^T/K^T computed in "dout-major" layout [1536, 1024] (12 tiles of 128
    partitions = 2 heads each), interleaved m-order so head 0 unblocks early.
  - V computed in s-major layout [1024, 12*65] with a constant-1 column per
    head, so each AV matmul also produces the softmax denominator row.
  - scores^T [k, q] per head; the two heads of a pair run as CONCURRENT
    64-row PE tiles (tile_position (0,0)/(64,0)) -> 2x scores throughput.
    exp on ScalarE with the 1/8 scale folded in.
  - AV: out_h^T [65, q] = V_ext^T @ exp^T accumulated over k tiles; row 64
    holds the softmax sums Z. Sums are staged to DRAM; 1/Z = exp(-ln Z)
    batched on ScalarE, lifted back and broadcast with ONE full-mode matmul
    per (pair, q-half) via a constant selector matrix (no PE mode switches).
  - Bias adds for V / proj are full-mode matmuls against zero-padded bias
    tiles (row 0 = bias) so the PE never drops into 32-row tiling mode.
  - The whole kernel is software-pipelined: pair g's scores/exp overlap
    pair g-1's AV and pair g+1's Q/K projection so the TensorEngine stays
    dense while ScalarE chews on exp (12.6M transcendentals).
  - Startup: weight pair-0 + x DMAs issue first; dummy matmuls warm the PE
    clock gate (HAM) while DMAs are in flight.
Compute dtype bf16 (fp32 PSUM accumulation).
"""

import sys

for _p in ("/opt/trn_rl_repo", "/root/.axon_site/_ro/trn_rl_repo"):
    if _p not in sys.path:
        sys.path.append(_p)

import numpy as np

import concourse.bacc as bacc
import concourse.mybir as mybir
import concourse.tile as tile
from concourse.bass_utils import run_bass_kernel_spmd
from concourse.masks import make_identity

N_CORES = 8
P = 128
S = 1024
D = 768
H = 12
HD = 64
ND = D // P            # 6 d_model chunks
NS = S // P            # 8 seq tiles
NM = (2 * D) // P      # 12 M-tiles over Q,K douts
SCALE = 1.0 / (HD ** 0.5)
BF = mybir.dt.bfloat16
F32 = mybir.dt.float32
AF = mybir.ActivationFunctionType
ALU = mybir.AluOpType

_cached = None


def _patch_act_tables():
    """Force every Exp/Ln activation onto the one table set that holds both
    (`natural_log_exp_and_others`), so the table is loaded once instead of
    thrashing between `exp_and_others` and the ln set on every head."""
    import concourse.bacc as _bacc
    if getattr(_bacc, "_act_tables_patched", False):
        return
    orig = _bacc.get_activation_tables

    def patched(arch):
        tables = dict(orig(arch))
        for name, fns in tables.items():
            if name != "natural_log_exp_and_others":
                tables[name] = fns - {AF.Exp, AF.Ln}
        return tables

    _bacc.get_activation_tables = patched
    _bacc._act_tables_patched = True


def _build():
    _patch_act_tables()
    nc = bacc.Bacc("TRN2", target_bir_lowering=False, debug=False,
                   enable_asserts=True, num_devices=N_CORES)

    x_ext = nc.dram_tensor("x", [S, D], F32, kind="ExternalInput").ap()
    wq_ext = nc.dram_tensor("W_qkv", [D, 3 * D], F32, kind="ExternalInput").ap()
    bq_ext = nc.dram_tensor("b_qkv", [1, 3 * D], F32, kind="ExternalInput").ap()
    wp_ext = nc.dram_tensor("W_proj", [D, D], F32, kind="ExternalInput").ap()
    bp_ext = nc.dram_tensor("b_proj", [1, D], F32, kind="ExternalInput").ap()
    out_ext = nc.dram_tensor("out", [S, D], F32, kind="ExternalOutput").ap()

    with tile.TileContext(nc) as tc:
        _body(nc, tc, x_ext, wq_ext, bq_ext, wp_ext, bp_ext, out_ext)

    nc.compile()
    return nc


def _body(nc, tc, x_ext, wq_ext, bq_ext, wp_ext, bp_ext, out_ext):
    from contextlib import ExitStack
    from concourse.tile import add_dep_helper
    with ExitStack() as ctx:
        persist = ctx.enter_context(tc.tile_pool(name="persist", bufs=1))
        yout = ctx.enter_context(tc.tile_pool(name="yout", bufs=2))
        ps_mm = ctx.enter_context(tc.tile_pool(name="ps_mm", bufs=2, space="PSUM"))

        # identity first (gpsimd affine_select is brief), then weight DMAs
        # on the same gpsimd (SWDGE) ring so the startup-critical Q/K pair-0
        # columns are in flight early.  All W loads are f32->bf16 cast-DMAs.
        ident = persist.tile([P, P], F32)
        make_identity(nc, ident)
        w_bf = persist.tile([P, ND, 3 * D], BF)
        # Q/K columns per-kc (6KB descriptors -- DMA cost is per descriptor,
        # so big contiguous per-partition runs matter more than pair order),
        # then V bias, V columns, proj bias, proj weights.  SWDGE ring FIFO
        # paces the transfers; no explicit deps.
        wqk_last = None
        for kc in range(ND):
            wqk_last = nc.gpsimd.dma_start(
                w_bf[:, kc, 0:2 * D], wq_ext[kc * P:(kc + 1) * P, 0:2 * D])
        wv_last = None
        for kc in range(ND):
            wv_last = nc.gpsimd.dma_start(
                w_bf[:, kc, 2 * D:3 * D],
                wq_ext[kc * P:(kc + 1) * P, 2 * D:3 * D])
            add_dep_helper(wv_last.ins, wqk_last.ins,
                           reason="V weights after QK weights")
        bv_pad = persist.tile([P, D], BF)
        bp_pad = persist.tile([P, D], BF)
        nc.vector.memset(bv_pad, 0.0)
        nc.vector.memset(bp_pad, 0.0)
        d = nc.gpsimd.dma_start(bv_pad[0:1, :], bq_ext[0:1, 2 * D:3 * D])
        add_dep_helper(d.ins, wqk_last.ins, reason="bv after QK")
        d = nc.gpsimd.dma_start(bp_pad[0:1, :], bp_ext[0:1, :])
        add_dep_helper(d.ins, wv_last.ins, reason="bp after V")
        wp_bf = persist.tile([P, ND, D], BF)     # row chunk g = head pair g
        for g in range(ND):
            wp_dma = nc.gpsimd.dma_start(wp_bf[:, g, :],
                                         wp_ext[g * P:(g + 1) * P, :])
            add_dep_helper(wp_dma.ins, wv_last.ins,
                           reason="proj weights after V weights")

        # ---- x loads on both HWDGE rings (f32, no cast; PE transposes read
        # f32 directly and the DVE drain casts to bf16) ----
        warm = persist.tile([P, 640], BF)
        nc.vector.memset(warm, 0.5)
        xT = persist.tile([P, ND, S], BF)
        with tc.tile_pool(name="xin", bufs=6) as xin, \
             tc.tile_pool(name="ps_tr", bufs=6, space="PSUM") as ps_tr:
            x_tiles = []
            bqkT = persist.tile([P, NM], F32)  # col m = b_qkv[m*128:(m+1)*128]
            for sb in range(NS):
                x_f = xin.tile([P, D], F32, tag="x_f")
                if sb < 6:
                    eng = nc.sync if sb % 2 == 0 else nc.scalar
                    eng.dma_start(x_f, x_ext[sb * P:(sb + 1) * P, :])
                else:
                    # sb6-7 are needed only after the first QKV groups; keep
                    # their descriptors out of the DMA queues until the
                    # startup-critical QK weight transfer has finished
                    d = nc.sync.dma_start(x_f, x_ext[sb * P:(sb + 1) * P, :])
                    add_dep_helper(d.ins, wqk_last.ins,
                                   reason="late x after QK weights")
                x_tiles.append(x_f)
                if sb == 3:  # QKV biases right behind the first four x tiles
                    for m in range(NM):
                        nc.sync.dma_start(bqkT[:, m:m + 1],
                                          bq_ext[0:1, m * P:(m + 1) * P])

            for sb in range(NS):
                x_f = x_tiles[sb]
                for kc in range(ND):
                    pt = ps_tr.tile([P, 512], F32, tag="ps_tr",
                                    name=f"tr{sb}_{kc}")
                    nc.tensor.transpose(pt[:, 0:P], x_f[:, kc * P:(kc + 1) * P],
                                        ident)
                    nc.vector.tensor_copy(xT[:, kc, sb * P:(sb + 1) * P],
                                          pt[:, 0:P])
        expp = ctx.enter_context(tc.tile_pool(name="expp", bufs=34))
        sums_p = ctx.enter_context(tc.tile_pool(name="sums", bufs=1))
        ps_sc = ctx.enter_context(tc.tile_pool(name="ps_sc", bufs=2, space="PSUM"))
        ps_av = ctx.enter_context(tc.tile_pool(name="ps_av", bufs=2, space="PSUM"))

        # row0_mat: row 0 all ones (full-mode bias broadcast lhsT)
        row0_mat = persist.tile([P, P], BF)
        nc.vector.memset(row0_mat, 0.0)
        nc.vector.memset(row0_mat[0:1, :], 1.0)
        # sel2: (row0, cols 0:64)=1 and (row32, cols 64:128)=1 -- lifts the
        # two 1/Z rows of rec2 to partition blocks 0:64/64:128 in one
        # full-mode matmul
        sel2 = persist.tile([P, P], BF)
        nc.vector.memset(sel2, 0.0)
        nc.vector.memset(sel2[0:1, 0:HD], 1.0)
        nc.vector.memset(sel2[32:33, HD:P], 1.0)
        vext = persist.tile([P, NS, H * 65], BF)
        for sb in range(NS):
            vd = vext[:, sb, :].rearrange("p (h c) -> p h c", c=65)
            nc.vector.memset(vd[:, :, 64:65], 1.0)
        # zt: softmax sums of the pair in flight (row 0/32 = head half,
        # slot=g%2); rec2: 1/Z rows.  zt is all-ones so Ln/Exp over rows
        # 0:33 gives 1.0 in the unused rows; sel2 zeros there kill them.
        zt = persist.tile([P, 2, S], BF)
        nc.vector.memset(zt, 1.0)   # ln(1)=0 -> 1/Z rows stay finite
        rec2 = persist.tile([P, 2, S], BF)
        nc.vector.memset(rec2, 0.0)

        qkT = persist.tile([P, NM, S], BF)
        aoT = persist.tile([P, ND, S], BF)   # paired attn out^T: pair g rows

        import itertools

        def gen_qkT(g):
            """Q^T/K^T tiles for pair g; yields each PE instruction (or None
            for non-PE work) so stage() can chain PE queue order.
            Order (Q,nh0),(K,nh0),(Q,nh1),(K,nh1) so the first scores
            matmuls of the pair unblock after two groups."""
            for m, nh in ((g, 0), (ND + g, 0), (g, 1), (ND + g, 1)):
                if True:
                    ps = ps_mm.tile([P, 512], F32, tag="ps_mm",
                                    name=f"qk{m}_{nh}")
                    for kc in range(ND):
                        mm = nc.tensor.matmul(
                            ps, w_bf[:, kc, m * P:(m + 1) * P],
                            xT[:, kc, nh * 512:(nh + 1) * 512],
                            start=(kc == 0), stop=(kc == ND - 1))
                        yield mm
                    nc.vector.tensor_scalar(
                        out=qkT[:, m, nh * 512:(nh + 1) * 512], in0=ps,
                        scalar1=bqkT[:, m:m + 1], scalar2=None, op0=ALU.add)
                    yield None

        def gen_v():
            """V in s-major with ones column per head."""
            for sb in range(NS):
                for c0, cn in ((0, 512), (512, 256)):
                    ps = ps_mm.tile([P, 512], F32, tag="ps_mm",
                                    name=f"v{sb}_{c0}")
                    for kc in range(ND):
                        mm = nc.tensor.matmul(
                            ps[:, :cn], xT[:, kc, sb * P:(sb + 1) * P],
                            w_bf[:, kc, 2 * D + c0:2 * D + c0 + cn],
                            start=(kc == 0), stop=False)
                        yield mm
                    mm = nc.tensor.matmul(ps[:, :cn], row0_mat,
                                          bv_pad[:, c0:c0 + cn],
                                          start=False, stop=True)
                    yield mm
                    h0 = c0 // HD
                    nh_h = cn // HD
                    vsrc = ps[:, :cn].rearrange("p (h c) -> p h c", c=HD)
                    vdst = vext[:, sb, :].rearrange("p (h c) -> p h c", c=65)
                    nc.vector.tensor_copy(vdst[:, h0:h0 + nh_h, 0:HD], vsrc)
                    yield None

        def av_epilogue(g, half, qh, po):
            """One PSUM read frees po; the Z row goes to zt (partition
            half, slot g%2); values relayed to aoT via a fast SBUF->SBUF
            bf16 copy."""
            h = 2 * g + half
            rows = slice(half * HD, (half + 1) * HD)
            qs = slice(qh * 512, (qh + 1) * 512)
            sst = sums_p.tile([65, 512], BF, tag="sst", name=f"sst{h}_{qh}",
                              bufs=6)
            nc.vector.tensor_copy(sst, po)
            nc.vector.tensor_copy(zt[32 * half:32 * half + 1, g % 2, qs],
                                  sst[64:65, :])
            nc.vector.tensor_copy(aoT[rows, g, qs], sst[0:64, :])
            return sst

        def finish_av(g, exps, interleaved, sums4):
            """Emit whatever AV work for pair g was not interleaved,
            kb-outer with both q-halves sharing each V weight load."""
            for half in range(2):
                todo = [qh for qh in range(2) if (half, qh) not in interleaved]
                if not todo:
                    continue
                h = 2 * g + half
                pos = {}
                for qh in todo:
                    pos[qh] = ps_av.tile([65, 512], F32, tag="ps_av",
                                         name=f"po{h}_{qh}x")
                for kb in range(NS):
                    for qh in todo:
                        nc.tensor.matmul(
                            pos[qh],
                            vext[:, kb, h * 65:(h + 1) * 65],
                            exps[half][kb][:, qh * 512:(qh + 1) * 512],
                            start=(kb == 0), stop=(kb == NS - 1))
                for qh in todo:
                    sums4[(half, qh)] = av_epilogue(g, half, qh, pos[qh])

        def stage(g, fillers, prev_exps, n_interleave=2, n_fill=6,
                  norm_g=None):
            """Scores+exp for pair g, with the previous pair's AV and other
            PE work threaded between the kb steps so the PE never starves
            while ScalarE chews on exp.  The 4 scores matmuls per kb are a
            dep-chained T0/T8/T0/T8 quartet so the two 64-row PE tiles run
            concurrently."""
            AVSET = ((0, 0), (0, 1), (1, 0))[:n_interleave]
            po = {}
            sums4 = {}
            e0 = []
            e1 = []
            acts = []
            # chain = force the exact PE queue order (AVs -> fillers ->
            # scores quartet per kb) so the T0/T8 quartet MMs stay adjacent
            # and pair up on the array.  Only for steady-state stages --
            # stage 0's fillers may stall on weight DMAs.
            chain = False
            pe_last = None

            def link(mm):
                nonlocal pe_last
                if chain and pe_last is not None:
                    add_dep_helper(mm.ins, pe_last.ins, reason="pe order")
                pe_last = mm
                return mm

            for kb in range(NS):
                if prev_exps is not None:
                    for half, qh in AVSET:
                        h = 2 * (g - 1) + half
                        if kb == 0:
                            po[(half, qh)] = ps_av.tile(
                                [65, 512], F32, tag="ps_av",
                                name=f"po{h}_{qh}")
                        link(nc.tensor.matmul(
                            po[(half, qh)],
                            vext[:, kb, h * 65:(h + 1) * 65],
                            prev_exps[half][kb][:, qh * 512:(qh + 1) * 512],
                            start=(kb == 0), stop=(kb == NS - 1)))
                for _ in range(n_fill):
                    r = next(fillers, StopIteration)
                    if r is StopIteration:
                        break
                    if r is not None:
                        link(r)
                ps0 = ps_sc.tile([P, S], F32, tag="ps_sc", name=f"sc0_{g}_{kb}")
                ps1 = ps_sc.tile([P, S], F32, tag="ps_sc", name=f"sc1_{g}_{kb}")
                kbs = slice(kb * P, (kb + 1) * P)
                for qh in range(2):
                    qs = slice(qh * 512, (qh + 1) * 512)
                    nc.tensor.matmul(ps0[:, qs], qkT[0:HD, ND + g, kbs],
                                     qkT[0:HD, g, qs], start=True, stop=True,
                                     tile_position=(0, 0))
                    nc.tensor.matmul(ps1[:, qs], qkT[HD:P, ND + g, kbs],
                                     qkT[HD:P, g, qs], start=True, stop=True,
                                     tile_position=(64, 0))
                t0 = expp.tile([P, S], BF, tag="expT", name=f"e0_{g}_{kb}")
                t1 = expp.tile([P, S], BF, tag="expT", name=f"e1_{g}_{kb}")
                a0 = nc.scalar.activation(t0, ps0, AF.Exp, scale=SCALE)
                a1 = nc.scalar.activation(t1, ps1, AF.Exp, scale=SCALE)
                acts.append((a0, a1))
                e0.append(t0)
                e1.append(t1)
            if prev_exps is not None:
                for half, qh in AVSET:
                    sums4[(half, qh)] = av_epilogue(g - 1, half, qh,
                                                    po[(half, qh)])
                finish_av(g - 1, prev_exps, set(AVSET), sums4)
            return (e0, e1)

        def norm_scalar(g):
            """Batched ln/exp for pair g's softmax sums (rows 0/32 of zt)."""
            slot = g % 2
            lnz = sums_p.tile([33, S], F32, tag="lnz", name=f"lnz{g}",
                              bufs=2)
            nc.scalar.activation(lnz, zt[0:33, slot, :], AF.Ln)
            nc.scalar.activation(rec2[0:33, slot, :], lnz, AF.Exp,
                                 scale=-1.0)

        def norm_pb(g, link_fn):
            """Lift the 1/Z rows to partition blocks with one full-mode
            matmul per q-half and scale aoT."""
            slot = g % 2
            for qh in range(2):
                qs = slice(qh * 512, (qh + 1) * 512)
                pb = ps_mm.tile([P, 512], F32, tag="ps_mm",
                                name=f"pb{g}_{qh}")
                link_fn(nc.tensor.matmul(pb, sel2, rec2[:, slot, qs],
                                         start=True, stop=True))
                nc.vector.tensor_mul(aoT[:, g, qs], aoT[:, g, qs], pb)

        # --- pipeline ---
        for _ in gen_qkT(0):
            pass
        fill0 = itertools.chain(gen_v(), gen_qkT(1))
        exps_prev = stage(0, fill0, None, n_fill=20)
        for _ in fill0:
            pass
        for g in range(1, ND):
            fill = gen_qkT(g + 1) if g + 1 < ND else iter(())
            exps_new = stage(g, fill, exps_prev)
            for _ in fill:
                pass
            exps_prev = exps_new
            # normalize the pair finished inside this stage; the scheduler
            # hides it under the next stage's compute
            norm_scalar(g - 1)
            norm_pb(g - 1, lambda mm: mm)
        sums5 = {}
        finish_av(ND - 1, exps_prev, set(), sums5)
        norm_scalar(ND - 1)
        norm_pb(ND - 1, lambda mm: mm)

        # ---- output projection (paired K=128 chunks); both column
        # halves accumulate in one 2-bank tile from the now-idle scores
        # pool, so each aoT weight load feeds N=768 of streaming ----
        for sb in range(NS):
            y_sb = yout.tile([P, D], F32, tag="y")
            ps = ps_sc.tile([P, S], F32, tag="ps_sc", name=f"prj{sb}")
            for g in range(ND):
                nc.tensor.matmul(ps[:, 0:512],
                                 aoT[:, g, sb * P:(sb + 1) * P],
                                 wp_bf[:, g, 0:512],
                                 start=(g == 0), stop=False)
                nc.tensor.matmul(ps[:, 512:768],
                                 aoT[:, g, sb * P:(sb + 1) * P],
                                 wp_bf[:, g, 512:768],
                                 start=(g == 0), stop=False)
            nc.tensor.matmul(ps[:, 0:512], row0_mat, bp_pad[:, 0:512],
                             start=False, stop=True)
            nc.tensor.matmul(ps[:, 512:768], row0_mat, bp_pad[:, 512:768],
                             start=False, stop=True)
            nc.vector.tensor_copy(y_sb, ps[:, 0:D])
            nc.sync.dma_start(out_ext[sb * P:(sb + 1) * P, :], y_sb)


def kernel(**inputs):
    global _cached
    x = np.ascontiguousarray(np.asarray(inputs["x"], dtype=np.float32))
    w_qkv = np.ascontiguousarray(np.asarray(inputs["W_qkv"], dtype=np.float32))
    b_qkv = np.ascontiguousarray(np.asarray(inputs["b_qkv"], dtype=np.float32)).reshape(1, -1)
    w_proj = np.ascontiguousarray(np.asarray(inputs["W_proj"], dtype=np.float32))
    b_proj = np.ascontiguousarray(np.asarray(inputs["b_proj"], dtype=np.float32)).reshape(1, -1)

    if _cached is None:
        _cached = _build()
    nc = _cached

    in_maps = [{"x": x[b], "W_qkv": w_qkv, "b_qkv": b_qkv,
                "W_proj": w_proj, "b_proj": b_proj} for b in range(N_CORES)]
    last_err = None
    for _attempt in range(3):
        try:
            res = run_bass_kernel_spmd(nc, in_maps,
                                       core_ids=list(range(N_CORES)))
            return np.stack([res.results[i]["out"] for i in range(N_CORES)],
                            axis=0)
        except Exception as e:  # transient NRT device errors happen rarely
            last_err = e
            import time
            time.sleep(2.0)
    raise last_err
